# revision 11
# baseline (speedup 1.0000x reference)
"""Trainium2 Bass kernel for BidirectionalCrossModalAttention (seq_len=1).

Math: with a single key, softmax == 1 exactly, so each MHA block reduces to
    mha(q, kv) = kv @ (w_out @ w_v).T + (w_out @ b_v + b_out)
i.e. one 1024x1024 matmul; gate matmuls fold the same way.  12 folded
1024x1024 matmuls per row + 4 sigmoid gates + 4 layernorms.

Precision plan (rel-err budget 2e-2):
  - value-critical matmuls W1 (v2t), W3 (a2t), W5 (a2v): fp16 x fp16
  - the other 9 matmuls: fp8-e4m3 DoubleRow (2x PE throughput), with
    per-matrix power-of-2 weight scales and x32 activation scale
  - elementwise/residual chain in fp16 (2x DVE throughput)
  - LN stats via ones-column fp8 DoubleRow matmuls; rstd by Quake-Newton
    rsqrt on DVE (keeps ACT on the sigmoid table set all kernel long)
  - outputs fp16, upconverted to f32 on the host

Layout: transposed — activations are [feature(->128 partitions, 8 chunks),
rows(->free)]; weights stationary, rows stream.  512-row blocks so
DoubleRow LDWEIGHTS (256 cols) hides under the 512-cycle matmuls.

Sharding: pure data parallelism — batch 32768 = 8 cores x 4096 rows; all
weights replicated; no collectives.
"""

import numpy as np
import ml_dtypes

import concourse.bass as bass
import concourse.tile as tile
from concourse import bacc, mybir
from concourse import bass_utils
from concourse.bass import ts

F32 = mybir.dt.float32
F16 = mybir.dt.float16
FP8 = mybir.dt.float8e4
U32 = mybir.dt.uint32
E4 = ml_dtypes.float8_e4m3fn
F16N = np.float16
AF = mybir.ActivationFunctionType
OP = mybir.AluOpType
PM = mybir.MatmulPerfMode

DIM = 1024
BATCH = 32768
NCORES = 8
R = BATCH // NCORES      # rows per core
C = DIM // 128           # feature chunks
EPS = 1e-5
NB = 512                 # rows per block
NBLK = R // NB
XS = 32.0                # fp8 activation scale

MAGICF = np.frombuffer(np.uint32(0x5F3759DF).tobytes(), dtype=np.float32)[0]

W16_NAMES = ["w1", "w3"]
W8_NAMES = ["w0", "m1", "w2", "m2", "w5", "w4", "m3", "g1bw1", "g2a", "g2b"]
VEC_IDX = {n: i for i, n in enumerate(
    ["p", "q", "r", "delta", "cm1", "cm2", "cm3t", "cga",
     "k0", "k2", "k4", "k5", "km1", "km2", "km3", "kg2",
     "g0", "b0", "g1", "b1", "g2", "b2"])}
NVEC = len(VEC_IDX)


def build_program(simple_ln=False):
    nc = bacc.Bacc("TRN2", target_bir_lowering=False, debug=False)

    xt16 = nc.dram_tensor("xt16", [128, C, R], F16, kind="ExternalInput").ap()
    xv16 = nc.dram_tensor("xv16", [128, C, R], F16, kind="ExternalInput").ap()
    xa16 = nc.dram_tensor("xa16", [128, C, R], F16, kind="ExternalInput").ap()
    xt8 = nc.dram_tensor("xt8", [128, C, R], FP8, kind="ExternalInput").ap()
    xv8 = nc.dram_tensor("xv8", [128, C, R], FP8, kind="ExternalInput").ap()
    xa8 = nc.dram_tensor("xa8", [128, C, R], FP8, kind="ExternalInput").ap()
    wd16 = {n: nc.dram_tensor(n, [128, C, DIM], F16, kind="ExternalInput").ap()
            for n in W16_NAMES}
    wd8 = {n: nc.dram_tensor(n, [128, C, DIM], FP8, kind="ExternalInput").ap()
           for n in W8_NAMES}
    vecs = nc.dram_tensor("vecs", [128, NVEC, C], F32, kind="ExternalInput").ap()
    ot = nc.dram_tensor("ot", [128, C, R], F16, kind="ExternalOutput").ap()
    ov = nc.dram_tensor("ov", [128, C, R], F16, kind="ExternalOutput").ap()
    oa = nc.dram_tensor("oa", [128, C, R], F16, kind="ExternalOutput").ap()

    with tile.TileContext(nc) as tc:
        _body(tc, xt16, xv16, xa16, xt8, xv8, xa8, wd16, wd8, vecs,
              ot, ov, oa, simple_ln)
    nc.compile()
    return nc


def _body(tc, xt16, xv16, xa16, xt8, xv8, xa8, wd16, wd8, vecs,
          ot, ov, oa, simple_ln):
    from contextlib import ExitStack
    with ExitStack() as ctx:
        _body_inner(ctx, tc, xt16, xv16, xa16, xt8, xv8, xa8, wd16, wd8,
                    vecs, ot, ov, oa, simple_ln)


def _body_inner(ctx, tc, xt16, xv16, xa16, xt8, xv8, xa8, wd16, wd8, vecs,
                ot, ov, oa, simple_ln):
    nc = tc.nc

    cst = ctx.enter_context(tc.tile_pool(name="cst", bufs=1))
    pmm = ctx.enter_context(tc.tile_pool(name="pmm", bufs=6, space="PSUM"))
    pst = ctx.enter_context(tc.tile_pool(name="pst", bufs=1, space="PSUM"))
    dram = ctx.enter_context(tc.tile_pool(name="dram", bufs=1, space="DRAM"))
    dsm = ctx.enter_context(tc.tile_pool(name="dsm", bufs=2, space="DRAM"))
    wp16 = ctx.enter_context(tc.tile_pool(name="wp16", bufs=2))
    wp8 = ctx.enter_context(tc.tile_pool(name="wp8", bufs=5))

    vec_sb = cst.tile([128, NVEC, C], F32, tag="vecs")
    nc.sync.dma_start(vec_sb[:], vecs[:])
    ones8 = cst.tile([128, 2, 16], FP8, tag="ones8")
    nc.vector.memset(ones8[:], 1.0)
    magic = cst.tile([1, NB], F32, tag="magic")
    nc.vector.memset(magic[:], float(MAGICF))

    v2t_d = dram.tile([128, C, R], F16, tag="v2t_d")
    te_d = dram.tile([128, C, R], F16, tag="te_d")
    a2t_d = dram.tile([128, C, R], F16, tag="a2t_d")
    sa16_d = dram.tile([128, C, R], F16, tag="sa16_d")
    sa8_d = dram.tile([128, C, R], FP8, tag="sa8_d")

    def vs(name, ci):
        return vec_sb[:, VEC_IDX[name], ci:ci + 1]

    def emit16(w_sb, rhs, consume):
        for oc in range(C):
            ps = pmm.tile([128, NB], F32, tag="mm")
            for kc in range(C):
                nc.tensor.matmul(ps[:], w_sb[:, kc, ts(oc, 128)],
                                 rhs[:, kc, :],
                                 start=(kc == 0), stop=(kc == C - 1))
            consume(oc, ps)

    def emit8(pairs, consume):
        total = (C // 2) * len(pairs)
        for oc in range(C):
            ps = pmm.tile([128, NB], F32, tag="mm")
            idx = 0
            for w_sb, rhs in pairs:
                for kp in range(0, C, 2):
                    nc.tensor.matmul(ps[:], w_sb[:, kp:kp + 2, ts(oc, 128)],
                                     rhs[:, kp:kp + 2, :],
                                     start=(idx == 0), stop=(idx == total - 1),
                                     perf_mode=PM.DoubleRow)
                    idx += 1
            consume(oc, ps)

    def act_evict(dst, bname, kname=None):
        def f(oc, ps):
            nc.scalar.activation(dst[:, oc, :], ps[:], AF.Identity,
                                 bias=vs(bname, oc),
                                 scale=(1.0 if kname is None else vs(kname, 0)))
        return f

    def stt_evict(dst, kname, other):
        def f(oc, ps):
            nc.vector.scalar_tensor_tensor(
                dst[:, oc, :], ps[:], vs(kname, 0), other[:, oc, :],
                OP.mult, OP.add)
        return f

    def sig_evict(dst, kname, cname):
        def f(oc, ps):
            nc.scalar.activation(dst[:, oc, :], ps[:], AF.Sigmoid,
                                 bias=vs(cname, oc), scale=vs(kname, 0))
        return f

    def make_stats_in(pool, y, key):
        yb8 = pool.tile([128, C, NB], FP8, tag=f"yb8{key}", bufs=2, name="yb8")
        nc.scalar.activation(yb8[:], y[:], AF.Copy, bias=0.0, scale=16.0)
        ysq8 = pool.tile([128, C, NB], FP8, tag=f"ys8{key}", bufs=2, name="ys8")
        nc.scalar.activation(ysq8[:], y[:], AF.Square, bias=0.0, scale=1.0)
        return yb8, ysq8

    def ln_stats(pool, yb8, ysq8, key):
        s1 = pst.tile([1, NB], F32, tag="s1", name="s1")
        for kp in range(0, C, 2):
            nc.tensor.matmul(s1[:], ones8[:, :, 0:1], yb8[:, kp:kp + 2, :],
                             start=(kp == 0), stop=(kp == C - 2),
                             perf_mode=PM.DoubleRow)
        s2 = pst.tile([1, NB], F32, tag="s2", name="s2")
        for kp in range(0, C, 2):
            nc.tensor.matmul(s2[:], ones8[:, :, 0:1], ysq8[:, kp:kp + 2, :],
                             start=(kp == 0), stop=(kp == C - 2),
                             perf_mode=PM.DoubleRow)
        sm = lambda nm: pool.tile([1, NB], F32, tag=nm, bufs=1, name=nm)
        nm_ = sm("negmu")
        nc.vector.tensor_scalar_mul(nm_[:], s1[:], -1.0 / (16.0 * DIM))
        t1 = sm("t1")
        nc.vector.tensor_mul(t1[:], nm_[:], nm_[:])
        u = sm("u")
        nc.vector.scalar_tensor_tensor(u[:], s2[:], 1.0 / DIM, t1[:],
                                       OP.mult, OP.subtract)
        # Quake rsqrt seed + one 2nd-order Householder step
        y0 = sm("y0")
        nc.vector.tensor_scalar(y0[:].bitcast(U32), u[:].bitcast(U32),
                                1, None, OP.logical_shift_right)
        nc.vector.tensor_tensor(y0[:].bitcast(U32), magic[:].bitcast(U32),
                                y0[:].bitcast(U32), OP.subtract)
        t2 = pool.tile([1, NB], F32, tag="u", bufs=1, name="t2")
        rstd16 = pool.tile([1, NB], F16, tag="rstd16", bufs=1, name="rstd16")
        nc.vector.tensor_mul(t1[:], y0[:], y0[:])
        nc.vector.tensor_mul(t1[:], t1[:], u[:])
        nc.vector.tensor_scalar(t2[:], t1[:], 0.375, -1.25, OP.mult, OP.add)
        nc.vector.tensor_mul(t2[:], t1[:], t2[:])
        nc.vector.tensor_scalar_add(t2[:], t2[:], 1.875)
        nc.vector.tensor_mul(rstd16[:], y0[:], t2[:])
        shift16 = pool.tile([1, NB], F16, tag="shift16", bufs=1, name="shift16")
        nc.vector.tensor_mul(shift16[:], nm_[:], rstd16[:])
        rd = dsm.tile([1, NB], F16, tag=f"rd{key}", bufs=2, name="rd")
        nc.sync.dma_start(rd[:], rstd16[:])
        sh = dsm.tile([1, NB], F16, tag=f"sh{key}", bufs=2, name="sh")
        nc.sync.dma_start(sh[:], shift16[:])
        ab = pool.tile([128, NB], F16, tag=f"ab{key}", bufs=2, name="ab")
        nc.sync.dma_start(ab[:], rd[:].to_broadcast((128, NB)))
        bb = pool.tile([128, NB], F16, tag=f"bb{key}", bufs=2, name="bb")
        nc.sync.dma_start(bb[:], sh[:].to_broadcast((128, NB)))
        return ab, bb

    def ln_apply(st, y, gname, bname):
        """In-place: y <- LN(y) (optionally * gamma + beta)."""
        ab, bb = st
        for ci in range(C):
            nc.vector.tensor_mul(y[:, ci, :], y[:, ci, :], ab[:])
            nc.vector.tensor_add(y[:, ci, :], y[:, ci, :], bb[:])
            if not simple_ln:
                nc.vector.tensor_scalar(y[:, ci, :], y[:, ci, :],
                                        vs(gname, ci), vs(bname, ci),
                                        OP.mult, OP.add)

    def load_w16(names):
        out = {}
        for n in names:
            w = wp16.tile([128, C, DIM], F16, tag="w16", name=n)
            nc.sync.dma_start(w[:], wd16[n][:])
            out[n] = w
        return out

    def load_w8(names):
        out = {}
        for n in names:
            w = wp8.tile([128, C, DIM], FP8, tag="w8", name=n)
            nc.sync.dma_start(w[:], wd8[n][:])
            out[n] = w
        return out

    bsl = lambda b: (slice(None), slice(None), ts(b, NB))

    def run_sweep(pool, mm_fn, out_dst, gname, bname, prefetch):
        hist = [None] * NBLK
        for b in range(NBLK + 2):
            if b == 1 and prefetch is not None:
                prefetch()
            if b < NBLK:
                y, yb8, ysq8 = mm_fn(b)
                hist[b] = [y, yb8, ysq8, None]
            if b >= 1 and b - 1 < NBLK:
                h = hist[b - 1]
                h[3] = ln_stats(pool, h[1], h[2], "")
            if b >= 2:
                h = hist[b - 2]
                ln_apply(h[3], h[0], gname, bname)
                nc.sync.dma_start(out_dst[bsl(b - 2)], h[0][:])
                hist[b - 2] = None

    # One shared activation pool across all 4 sweeps: tags are reused
    # sweep-to-sweep so there is no pool-close drain barrier between
    # sweeps - the next sweep's first blocks overlap the previous tail.
    ap = ctx.enter_context(tc.tile_pool(name="apool", bufs=2))

    # ---- Sweep 1: text stage 1 -> te_d, v2t_d ---------------------------
    w16 = load_w16(["w1"])
    w8 = load_w8(["w0", "m1"])
    nxt16, nxt8 = {}, {}

    def s1_mm(b):
        t16 = ap.tile([128, C, NB], F16, tag="x16", bufs=1, name="t16")
        nc.sync.dma_start(t16[:], xt16[bsl(b)])
        t8 = ap.tile([128, C, NB], FP8, tag="x8b", bufs=1, name="t8")
        nc.sync.dma_start(t8[:], xt8[bsl(b)])
        v8 = ap.tile([128, C, NB], FP8, tag="x8a", bufs=2, name="v8")
        nc.sync.dma_start(v8[:], xv8[bsl(b)])
        P = ap.tile([128, C, NB], F16, tag="bige", bufs=2, name="P")
        emit16(w16["w1"], t16, act_evict(P, "p"))
        nc.sync.dma_start(v2t_d[bsl(b)], P[:])
        y = ap.tile([128, C, NB], F16, tag="y", bufs=3, name="y")
        emit8([(w8["w0"], v8)], stt_evict(y, "k0", P))
        g = ap.tile([128, C, NB], F16, tag="g", bufs=1, name="g")
        emit8([(w8["m1"], t8)], sig_evict(g, "km1", "cm1"))
        nc.vector.tensor_mul(y[:], g[:], y[:])
        nc.vector.tensor_add(y[:], y[:], t16[:])
        yb8, ysq8 = make_stats_in(ap, y, "")
        return y, yb8, ysq8

    def pf1():
        nxt16.update(load_w16(["w3"]))
        nxt8.update(load_w8(["w2", "m2"]))

    run_sweep(ap, s1_mm, te_d, "g0", "b0", pf1)
    w16, w8 = nxt16, nxt8
    nxt16, nxt8 = {}, {}

    # ---- Sweep 2: text stage 2 -> ot, a2t_d -----------------------------
    def s2_mm(b):
        te16 = ap.tile([128, C, NB], F16, tag="x16", bufs=1, name="te16")
        nc.sync.dma_start(te16[:], te_d[bsl(b)])
        te8 = ap.tile([128, C, NB], FP8, tag="sa8", bufs=2, name="te8")
        nc.scalar.activation(te8[:], te16[:], AF.Copy, bias=0.0, scale=XS)
        a8 = ap.tile([128, C, NB], FP8, tag="x8a", bufs=2, name="a8")
        nc.sync.dma_start(a8[:], xa8[bsl(b)])
        Q = ap.tile([128, C, NB], F16, tag="bige", bufs=2, name="Q")
        emit16(w16["w3"], te16, act_evict(Q, "q"))
        nc.sync.dma_start(a2t_d[bsl(b)], Q[:])
        y = ap.tile([128, C, NB], F16, tag="y", bufs=3, name="y")
        emit8([(w8["w2"], a8)], stt_evict(y, "k2", Q))
        g = ap.tile([128, C, NB], F16, tag="g", bufs=1, name="g")
        emit8([(w8["m2"], te8)], sig_evict(g, "km2", "cm2"))
        nc.vector.tensor_mul(y[:], g[:], y[:])
        nc.vector.tensor_add(y[:], y[:], te16[:])
        yb8, ysq8 = make_stats_in(ap, y, "")
        return y, yb8, ysq8

    def pf2():
        nxt8.update(load_w8(["w5", "w4", "m3", "g1bw1"]))

    run_sweep(ap, s2_mm, ot, "g0", "b0", pf2)
    w8 = nxt8
    nxt8 = {}

    # ---- Sweep 3: vision -> ov, a2v_d -----------------------------------
    def s3_mm(b):
        v16 = ap.tile([128, C, NB], F16, tag="x16", bufs=1, name="v16")
        nc.sync.dma_start(v16[:], xv16[bsl(b)])
        v8 = ap.tile([128, C, NB], FP8, tag="x8a", bufs=2, name="v8")
        nc.sync.dma_start(v8[:], xv8[bsl(b)])
        t8 = ap.tile([128, C, NB], FP8, tag="x8b", bufs=1, name="t8")
        nc.sync.dma_start(t8[:], xt8[bsl(b)])
        a8 = ap.tile([128, C, NB], FP8, tag="x8c", bufs=1, name="a8")
        nc.sync.dma_start(a8[:], xa8[bsl(b)])
        P = ap.tile([128, C, NB], F16, tag="bige", bufs=2, name="P")
        nc.sync.dma_start(P[:], v2t_d[bsl(b)])
        Rt = ap.tile([128, C, NB], F16, tag="bige", bufs=2, name="Rt")
        emit8([(w8["w5"], v8)], act_evict(Rt, "r", "k5"))
        y = ap.tile([128, C, NB], F16, tag="y", bufs=3, name="y")
        emit8([(w8["w4"], a8)], stt_evict(y, "k4", Rt))
        g = ap.tile([128, C, NB], F16, tag="g", bufs=1, name="g")
        emit8([(w8["m3"], v8), (w8["g1bw1"], t8)],
              sig_evict(g, "km3", "cm3t"))
        # audio's gated sum sa = a2t + a2v + delta: computed here where PE
        # has headroom, streamed to DRAM for sweep 4
        Q = ap.tile([128, C, NB], F16, tag="sa", bufs=2, name="Q")
        nc.sync.dma_start(Q[:], a2t_d[bsl(b)])
        sa = ap.tile([128, C, NB], F16, tag="sa", bufs=2, name="sa")
        for ci in range(C):
            nc.vector.scalar_tensor_tensor(
                sa[:, ci, :], Q[:, ci, :], vs("delta", ci), Rt[:, ci, :],
                OP.add, OP.add)
        nc.sync.dma_start(sa16_d[bsl(b)], sa[:])
        sa8 = ap.tile([128, C, NB], FP8, tag="sa8", bufs=2, name="sa8")
        nc.scalar.activation(sa8[:], sa[:], AF.Copy, bias=0.0, scale=XS)
        nc.sync.dma_start(sa8_d[bsl(b)], sa8[:])
        nc.vector.tensor_add(y[:], y[:], P[:])
        nc.vector.tensor_mul(y[:], g[:], y[:])
        nc.vector.tensor_add(y[:], y[:], v16[:])
        yb8, ysq8 = make_stats_in(ap, y, "")
        return y, yb8, ysq8

    def pf3():
        nxt8.update(load_w8(["g2a", "g2b"]))

    run_sweep(ap, s3_mm, ov, "g1", "b1", pf3)
    w8 = nxt8

    # ---- Sweep 4: audio -> oa -------------------------------------------
    def s4_mm(b):
        a16 = ap.tile([128, C, NB], F16, tag="x16", bufs=1, name="a16")
        nc.sync.dma_start(a16[:], xa16[bsl(b)])
        a8 = ap.tile([128, C, NB], FP8, tag="x8a", bufs=2, name="a8")
        nc.sync.dma_start(a8[:], xa8[bsl(b)])
        sa = ap.tile([128, C, NB], F16, tag="sa", bufs=2, name="sa")
        nc.sync.dma_start(sa[:], sa16_d[bsl(b)])
        sa8 = ap.tile([128, C, NB], FP8, tag="sa8", bufs=2, name="sa8")
        nc.sync.dma_start(sa8[:], sa8_d[bsl(b)])
        g = ap.tile([128, C, NB], F16, tag="g", bufs=1, name="g")
        emit8([(w8["g2a"], a8), (w8["g2b"], sa8)],
              sig_evict(g, "kg2", "cga"))
        y = ap.tile([128, C, NB], F16, tag="y", bufs=3, name="y")
        nc.vector.tensor_mul(y[:], g[:], sa[:])
        nc.vector.tensor_add(y[:], y[:], a16[:])
        yb8, ysq8 = make_stats_in(ap, y, "")
        return y, yb8, ysq8

    run_sweep(ap, s4_mm, oa, "g2", "b2", None)


# ---------------------------------------------------------------------------
# Host side
# ---------------------------------------------------------------------------

def _pack_act(x, dtype, scale=1.0):
    """[rows, 1024] f32 -> [128, C, rows] (transposed, chunked)."""
    r = x.shape[0]
    v = x.T.reshape(C, 128, r).transpose(1, 0, 2)
    if scale != 1.0:
        v = np.clip(v * scale, -240.0, 240.0)
    return np.ascontiguousarray(v).astype(dtype)


def _pack_w(m, dtype, scale=1.0):
    """W [1024(out), 1024(in)] -> lhsT [128, C(kc), 1024(out)]."""
    v = m.reshape(DIM, C, 128).transpose(2, 1, 0)
    if scale != 1.0:
        v = np.clip(v * scale, -240.0, 240.0)
    return np.ascontiguousarray(v).astype(dtype)


def _unpack_out(o):
    """[128, C, rows] f16 -> [rows, 1024] f32."""
    r = o.shape[2]
    return np.ascontiguousarray(
        o.transpose(1, 0, 2).astype(np.float32).reshape(DIM, r).T)


def _pow2_scale(m, target=192.0):
    return float(2.0 ** np.floor(np.log2(target / np.abs(m).max())))


_PROG = {}


def _get_prog(simple_ln):
    if simple_ln not in _PROG:
        _PROG[simple_ln] = build_program(simple_ln=simple_ln)
    return _PROG[simple_ln]


def fold_weights(mha_w_in, mha_b_in, mha_w_out, mha_b_out, gate_w, gate_b):
    W, c = [], []
    for i in range(6):
        w_v = mha_w_in[i][2 * DIM:3 * DIM]
        b_v = mha_b_in[i][2 * DIM:3 * DIM]
        W.append(mha_w_out[i] @ w_v)
        c.append(mha_w_out[i] @ b_v + mha_b_out[i])
    Ga = [gate_w[j][:, :DIM] for j in range(3)]
    Gb = [gate_w[j][:, DIM:] for j in range(3)]
    mats = {
        "w1": W[1], "w3": W[3], "w5": W[5],
        "w0": W[0], "w2": W[2], "w4": W[4],
        "m1": Ga[0] + Gb[0] @ W[1],
        "m2": Ga[0] + Gb[0] @ W[3],
        "m3": Ga[1] + Gb[1] @ W[5],
        "g1bw1": Gb[1] @ W[1],
        "g2a": Ga[2], "g2b": Gb[2],
    }
    cvecs = {
        "p": c[0] + c[1],
        "q": c[2] + c[3],
        "r": c[4] + c[5] - c[0],
        "delta": c[0] - c[2] - c[4],
        "cm1": gate_b[0] + Gb[0] @ c[1],
        "cm2": gate_b[0] + Gb[0] @ c[3],
        "cm3t": gate_b[1] + Gb[1] @ c[5] + Gb[1] @ c[1],
        "cga": gate_b[2],
    }
    return mats, cvecs


LAST_EXEC_TIME_NS = None


def timed_run(inputs):
    """Re-run the kernel with NTFF tracing; returns HW exec time in ns."""
    kernel(**inputs, _trace=True)
    return LAST_EXEC_TIME_NS


def kernel(text, vision, audio, mha_w_in, mha_b_in, mha_w_out, mha_b_out,
           gate_w, gate_b, ln_scale, ln_bias, _trace=False):
    f32 = lambda a: np.asarray(a, dtype=np.float32)
    text, vision, audio = f32(text), f32(vision), f32(audio)
    mha_w_in, mha_b_in = f32(mha_w_in), f32(mha_b_in)
    mha_w_out, mha_b_out = f32(mha_w_out), f32(mha_b_out)
    gate_w, gate_b = f32(gate_w), f32(gate_b)
    ln_scale, ln_bias = f32(ln_scale), f32(ln_bias)

    simple_ln = bool(np.all(ln_scale == 1.0) and np.all(ln_bias == 0.0))
    nc = _get_prog(simple_ln)

    mats, cvecs = fold_weights(mha_w_in, mha_b_in, mha_w_out, mha_b_out,
                               gate_w, gate_b)
    # fp8 weight scales (shared within accumulation groups)
    sc = {n: _pow2_scale(mats[n]) for n in W8_NAMES}
    s3 = min(sc["m3"], sc["g1bw1"])
    sc["m3"] = sc["g1bw1"] = s3
    s4 = min(sc["g2a"], sc["g2b"])
    sc["g2a"] = sc["g2b"] = s4

    wdev = {n: _pack_w(mats[n], F16N) for n in W16_NAMES}
    wdev.update({n: _pack_w(mats[n], E4, sc[n]) for n in W8_NAMES})

    V = np.zeros((NVEC, DIM), np.float32)
    for n in ("p", "q", "r", "delta", "cm1", "cm2", "cm3t", "cga"):
        V[VEC_IDX[n]] = cvecs[n]
    for n, kn in [("w0", "k0"), ("w2", "k2"), ("w4", "k4"), ("w5", "k5"),
                  ("m1", "km1"), ("m2", "km2"), ("m3", "km3"), ("g2a", "kg2")]:
        V[VEC_IDX[kn]] = 1.0 / (XS * sc[n])
    for j, (gn, bn) in enumerate([("g0", "b0"), ("g1", "b1"), ("g2", "b2")]):
        V[VEC_IDX[gn]] = ln_scale[j]
        V[VEC_IDX[bn]] = ln_bias[j]
    vecs_dev = np.ascontiguousarray(
        V.reshape(NVEC, C, 128).transpose(2, 0, 1)).astype(np.float32)

    in_maps = []
    for cid in range(NCORES):
        sl = slice(cid * R, (cid + 1) * R)
        in_maps.append({
            "xt16": _pack_act(text[sl], F16N),
            "xv16": _pack_act(vision[sl], F16N),
            "xa16": _pack_act(audio[sl], F16N),
            "xt8": _pack_act(text[sl], E4, XS),
            "xv8": _pack_act(vision[sl], E4, XS),
            "xa8": _pack_act(audio[sl], E4, XS),
            "vecs": vecs_dev,
            **wdev,
        })

    # The device occasionally throws a transient NRT_EXEC_UNIT_UNRECOVERABLE
    # on the first execute; retry a couple of times before giving up.
    last_err = None
    for attempt in range(3):
        try:
            res = bass_utils.run_bass_kernel_spmd(
                nc, in_maps, core_ids=list(range(NCORES)), trace=_trace)
            break
        except Exception as e:
            last_err = e
            import time as _time
            _time.sleep(5)
    else:
        raise last_err
    if _trace:
        global LAST_EXEC_TIME_NS
        LAST_EXEC_TIME_NS = res.exec_time_ns
        if res.instructions_and_trace:
            print("trace:", res.instructions_and_trace[1])

    outs = {k: np.empty((BATCH, DIM), np.float32) for k in ("ot", "ov", "oa")}
    for cid in range(NCORES):
        sl = slice(cid * R, (cid + 1) * R)
        for k in outs:
            outs[k][sl] = _unpack_out(res.results[cid][k])
    return (outs["ot"], outs["ov"], outs["oa"])


# revision 12
# speedup vs baseline: 1.3585x; 1.3585x over previous
"""Trainium2 Bass kernel for BidirectionalCrossModalAttention (seq_len=1).

Math: with a single key, softmax == 1 exactly, so each MHA block reduces to
    mha(q, kv) = kv @ (w_out @ w_v).T + (w_out @ b_v + b_out)
i.e. one 1024x1024 matmul; gate matmuls fold the same way.  12 folded
1024x1024 matmuls per row + 4 sigmoid gates + 4 layernorms.

Precision plan (rel-err budget 2e-2):
  - value-critical matmuls W1 (v2t), W3 (a2t), W5 (a2v): fp16 x fp16
  - the other 9 matmuls: fp8-e4m3 DoubleRow (2x PE throughput), with
    per-matrix power-of-2 weight scales and x32 activation scale
  - elementwise/residual chain in fp16 (2x DVE throughput)
  - LN stats via ones-column fp8 DoubleRow matmuls; rstd by Quake-Newton
    rsqrt on DVE (keeps ACT on the sigmoid table set all kernel long)
  - outputs fp16, upconverted to f32 on the host

Layout: transposed — activations are [feature(->128 partitions, 8 chunks),
rows(->free)]; weights stationary, rows stream.  512-row blocks so
DoubleRow LDWEIGHTS (256 cols) hides under the 512-cycle matmuls.

Sharding: pure data parallelism — batch 32768 = 8 cores x 4096 rows; all
weights replicated; no collectives.
"""

import numpy as np
import ml_dtypes

import concourse.bass as bass
import concourse.tile as tile
from concourse import bacc, mybir
from concourse import bass_utils
from concourse.bass import ts

F32 = mybir.dt.float32
F16 = mybir.dt.float16
FP8 = mybir.dt.float8e4
U32 = mybir.dt.uint32
E4 = ml_dtypes.float8_e4m3fn
F16N = np.float16
AF = mybir.ActivationFunctionType
OP = mybir.AluOpType
PM = mybir.MatmulPerfMode

DIM = 1024
BATCH = 32768
NCORES = 8
R = BATCH // NCORES      # rows per core
C = DIM // 128           # feature chunks
EPS = 1e-5
NB = 512                 # rows per block
NBLK = R // NB
XS = 32.0                # fp8 activation scale

MAGICF = np.frombuffer(np.uint32(0x5F3759DF).tobytes(), dtype=np.float32)[0]

W16_NAMES = ["w1", "w3"]
W8_NAMES = ["w0", "m1", "w2", "m2", "w5", "w4", "m3", "g1bw1", "g2a", "g2b"]
VEC_IDX = {n: i for i, n in enumerate(
    ["p", "q", "r", "delta", "cm1", "cm2", "cm3t", "cga",
     "k0", "k2", "k4", "k5", "km1", "km2", "km3", "kg2",
     "g0", "b0", "g1", "b1", "g2", "b2"])}
NVEC = len(VEC_IDX)


def build_program(simple_ln=False):
    nc = bacc.Bacc("TRN2", target_bir_lowering=False, debug=False)

    xt16 = nc.dram_tensor("xt16", [128, C, R], F16, kind="ExternalInput").ap()
    xv16 = nc.dram_tensor("xv16", [128, C, R], F16, kind="ExternalInput").ap()
    xa16 = nc.dram_tensor("xa16", [128, C, R], F16, kind="ExternalInput").ap()
    xt8 = nc.dram_tensor("xt8", [128, C, R], FP8, kind="ExternalInput").ap()
    xv8 = nc.dram_tensor("xv8", [128, C, R], FP8, kind="ExternalInput").ap()
    xa8 = nc.dram_tensor("xa8", [128, C, R], FP8, kind="ExternalInput").ap()
    wd16 = {n: nc.dram_tensor(n, [128, C, DIM], F16, kind="ExternalInput").ap()
            for n in W16_NAMES}
    wd8 = {n: nc.dram_tensor(n, [128, C, DIM], FP8, kind="ExternalInput").ap()
           for n in W8_NAMES}
    vecs = nc.dram_tensor("vecs", [128, NVEC, C], F32, kind="ExternalInput").ap()
    ot = nc.dram_tensor("ot", [128, C, R], F16, kind="ExternalOutput").ap()
    ov = nc.dram_tensor("ov", [128, C, R], F16, kind="ExternalOutput").ap()
    oa = nc.dram_tensor("oa", [128, C, R], F16, kind="ExternalOutput").ap()

    with tile.TileContext(nc) as tc:
        _body(tc, xt16, xv16, xa16, xt8, xv8, xa8, wd16, wd8, vecs,
              ot, ov, oa, simple_ln)
    nc.compile()
    return nc


def _body(tc, xt16, xv16, xa16, xt8, xv8, xa8, wd16, wd8, vecs,
          ot, ov, oa, simple_ln):
    from contextlib import ExitStack
    with ExitStack() as ctx:
        _body_inner(ctx, tc, xt16, xv16, xa16, xt8, xv8, xa8, wd16, wd8,
                    vecs, ot, ov, oa, simple_ln)


def _body_inner(ctx, tc, xt16, xv16, xa16, xt8, xv8, xa8, wd16, wd8, vecs,
                ot, ov, oa, simple_ln):
    nc = tc.nc

    cst = ctx.enter_context(tc.tile_pool(name="cst", bufs=1))
    pmm = ctx.enter_context(tc.tile_pool(name="pmm", bufs=6, space="PSUM"))
    pst = ctx.enter_context(tc.tile_pool(name="pst", bufs=1, space="PSUM"))
    dram = ctx.enter_context(tc.tile_pool(name="dram", bufs=1, space="DRAM"))
    dsm = ctx.enter_context(tc.tile_pool(name="dsm", bufs=2, space="DRAM"))
    wp16 = ctx.enter_context(tc.tile_pool(name="wp16", bufs=2))
    wp8 = ctx.enter_context(tc.tile_pool(name="wp8", bufs=5))

    vec_sb = cst.tile([128, NVEC, C], F32, tag="vecs")
    nc.sync.dma_start(vec_sb[:], vecs[:])
    ones8 = cst.tile([128, 2, 16], FP8, tag="ones8")
    nc.vector.memset(ones8[:], 1.0)
    magic = cst.tile([1, NB], F32, tag="magic")
    nc.vector.memset(magic[:], float(MAGICF))

    v2t_d = dram.tile([128, C, R], F16, tag="v2t_d")
    te_d = dram.tile([128, C, R], F16, tag="te_d")
    a2t_d = dram.tile([128, C, R], F16, tag="a2t_d")
    sa16_d = dram.tile([128, C, R], F16, tag="sa16_d")
    sa8_d = dram.tile([128, C, R], FP8, tag="sa8_d")

    def vs(name, ci):
        return vec_sb[:, VEC_IDX[name], ci:ci + 1]

    def emit16(w_sb, rhs, consume):
        for oc in range(C):
            ps = pmm.tile([128, NB], F32, tag="mm")
            for kc in range(C):
                nc.tensor.matmul(ps[:], w_sb[:, kc, ts(oc, 128)],
                                 rhs[:, kc, :],
                                 start=(kc == 0), stop=(kc == C - 1))
            consume(oc, ps)

    def emit8(pairs, consume):
        total = (C // 2) * len(pairs)
        for oc in range(C):
            ps = pmm.tile([128, NB], F32, tag="mm")
            idx = 0
            for w_sb, rhs in pairs:
                for kp in range(0, C, 2):
                    nc.tensor.matmul(ps[:], w_sb[:, kp:kp + 2, ts(oc, 128)],
                                     rhs[:, kp:kp + 2, :],
                                     start=(idx == 0), stop=(idx == total - 1),
                                     perf_mode=PM.DoubleRow)
                    idx += 1
            consume(oc, ps)

    def act_evict(dst, bname, kname=None):
        def f(oc, ps):
            nc.scalar.activation(dst[:, oc, :], ps[:], AF.Identity,
                                 bias=vs(bname, oc),
                                 scale=(1.0 if kname is None else vs(kname, 0)))
        return f

    def stt_evict(dst, kname, other):
        def f(oc, ps):
            nc.vector.scalar_tensor_tensor(
                dst[:, oc, :], ps[:], vs(kname, 0), other[:, oc, :],
                OP.mult, OP.add)
        return f

    def sig_evict(dst, kname, cname):
        def f(oc, ps):
            nc.scalar.activation(dst[:, oc, :], ps[:], AF.Sigmoid,
                                 bias=vs(cname, oc), scale=vs(kname, 0))
        return f

    def make_stats_in(pool, y, key):
        yb8 = pool.tile([128, C, NB], FP8, tag=f"yb8{key}", bufs=2, name="yb8")
        nc.scalar.activation(yb8[:], y[:], AF.Copy, bias=0.0, scale=16.0)
        ysq8 = pool.tile([128, C, NB], FP8, tag=f"ys8{key}", bufs=2, name="ys8")
        nc.scalar.activation(ysq8[:], y[:], AF.Square, bias=0.0, scale=1.0)
        return yb8, ysq8

    def ln_stats(pool, yb8, ysq8, key):
        s1 = pst.tile([1, NB], F32, tag="s1", name="s1")
        for kp in range(0, C, 2):
            nc.tensor.matmul(s1[:], ones8[:, :, 0:1], yb8[:, kp:kp + 2, :],
                             start=(kp == 0), stop=(kp == C - 2),
                             perf_mode=PM.DoubleRow)
        s2 = pst.tile([1, NB], F32, tag="s2", name="s2")
        for kp in range(0, C, 2):
            nc.tensor.matmul(s2[:], ones8[:, :, 0:1], ysq8[:, kp:kp + 2, :],
                             start=(kp == 0), stop=(kp == C - 2),
                             perf_mode=PM.DoubleRow)
        sm = lambda nm: pool.tile([1, NB], F32, tag=nm, bufs=1, name=nm)
        nm_ = sm("negmu")
        nc.vector.tensor_scalar_mul(nm_[:], s1[:], -1.0 / (16.0 * DIM))
        t1 = sm("t1")
        nc.vector.tensor_mul(t1[:], nm_[:], nm_[:])
        u = sm("u")
        nc.vector.scalar_tensor_tensor(u[:], s2[:], 1.0 / DIM, t1[:],
                                       OP.mult, OP.subtract)
        # Quake rsqrt seed + one 2nd-order Householder step
        y0 = sm("y0")
        nc.vector.tensor_scalar(y0[:].bitcast(U32), u[:].bitcast(U32),
                                1, None, OP.logical_shift_right)
        nc.vector.tensor_tensor(y0[:].bitcast(U32), magic[:].bitcast(U32),
                                y0[:].bitcast(U32), OP.subtract)
        t2 = pool.tile([1, NB], F32, tag="u", bufs=1, name="t2")
        rstd16 = pool.tile([1, NB], F16, tag="rstd16", bufs=1, name="rstd16")
        nc.vector.tensor_mul(t1[:], y0[:], y0[:])
        nc.vector.tensor_mul(t1[:], t1[:], u[:])
        nc.vector.tensor_scalar(t2[:], t1[:], 0.375, -1.25, OP.mult, OP.add)
        nc.vector.tensor_mul(t2[:], t1[:], t2[:])
        nc.vector.tensor_scalar_add(t2[:], t2[:], 1.875)
        nc.vector.tensor_mul(rstd16[:], y0[:], t2[:])
        shift16 = pool.tile([1, NB], F16, tag="shift16", bufs=1, name="shift16")
        nc.vector.tensor_mul(shift16[:], nm_[:], rstd16[:])
        rd = dsm.tile([1, NB], F16, tag=f"rd{key}", bufs=2, name="rd")
        nc.sync.dma_start(rd[:], rstd16[:])
        sh = dsm.tile([1, NB], F16, tag=f"sh{key}", bufs=2, name="sh")
        nc.sync.dma_start(sh[:], shift16[:])
        ab = pool.tile([128, NB], F16, tag=f"ab{key}", bufs=2, name="ab")
        nc.sync.dma_start(ab[:], rd[:].to_broadcast((128, NB)))
        bb = pool.tile([128, NB], F16, tag=f"bb{key}", bufs=2, name="bb")
        nc.sync.dma_start(bb[:], sh[:].to_broadcast((128, NB)))
        return ab, bb

    def ln_apply(st, y, gname, bname):
        """In-place: y <- LN(y) (optionally * gamma + beta)."""
        ab, bb = st
        for ci in range(C):
            nc.vector.tensor_mul(y[:, ci, :], y[:, ci, :], ab[:])
            nc.vector.tensor_add(y[:, ci, :], y[:, ci, :], bb[:])
            if not simple_ln:
                nc.vector.tensor_scalar(y[:, ci, :], y[:, ci, :],
                                        vs(gname, ci), vs(bname, ci),
                                        OP.mult, OP.add)

    def load_w16(names):
        out = {}
        for n in names:
            w = wp16.tile([128, C, DIM], F16, tag="w16", name=n)
            nc.sync.dma_start(w[:], wd16[n][:])
            out[n] = w
        return out

    def load_w8(names):
        out = {}
        for n in names:
            w = wp8.tile([128, C, DIM], FP8, tag="w8", name=n)
            nc.sync.dma_start(w[:], wd8[n][:])
            out[n] = w
        return out

    bsl = lambda b: (slice(None), slice(None), ts(b, NB))

    def run_sweep(pool, mm_fn, out_dst, gname, bname, prefetch):
        hist = [None] * NBLK
        for b in range(NBLK + 2):
            if b == 1 and prefetch is not None:
                prefetch()
            if b < NBLK:
                y, yb8, ysq8 = mm_fn(b)
                hist[b] = [y, yb8, ysq8, None]
            if b >= 1 and b - 1 < NBLK:
                h = hist[b - 1]
                h[3] = ln_stats(pool, h[1], h[2], "")
            if b >= 2:
                h = hist[b - 2]
                ln_apply(h[3], h[0], gname, bname)
                nc.sync.dma_start(out_dst[bsl(b - 2)], h[0][:])
                hist[b - 2] = None

    # One shared activation pool across all 4 sweeps: tags are reused
    # sweep-to-sweep so there is no pool-close drain barrier between
    # sweeps - the next sweep's first blocks overlap the previous tail.
    ap = ctx.enter_context(tc.tile_pool(name="apool", bufs=2))

    # ---- Sweep 1: text stage 1 -> te_d, v2t_d ---------------------------
    w16 = load_w16(["w1"])
    w8 = load_w8(["w0", "m1"])
    nxt16, nxt8 = {}, {}

    def s1_mm(b):
        t16 = ap.tile([128, C, NB], F16, tag="x16", bufs=2, name="t16")
        nc.sync.dma_start(t16[:], xt16[bsl(b)])
        t8 = ap.tile([128, C, NB], FP8, tag="sa8", bufs=2, name="t8")
        nc.sync.dma_start(t8[:], xt8[bsl(b)])
        v8 = ap.tile([128, C, NB], FP8, tag="x8a", bufs=2, name="v8")
        nc.sync.dma_start(v8[:], xv8[bsl(b)])
        P = ap.tile([128, C, NB], F16, tag="bige", bufs=2, name="P")
        emit16(w16["w1"], t16, act_evict(P, "p"))
        nc.sync.dma_start(v2t_d[bsl(b)], P[:])
        y = ap.tile([128, C, NB], F16, tag="y", bufs=3, name="y")
        emit8([(w8["w0"], v8)], stt_evict(y, "k0", P))
        g = ap.tile([128, C, NB], F16, tag="g", bufs=1, name="g")
        emit8([(w8["m1"], t8)], sig_evict(g, "km1", "cm1"))
        nc.vector.tensor_mul(y[:], g[:], y[:])
        nc.vector.tensor_add(y[:], y[:], t16[:])
        yb8, ysq8 = make_stats_in(ap, y, "")
        return y, yb8, ysq8

    def pf1():
        nxt16.update(load_w16(["w3"]))
        nxt8.update(load_w8(["w2", "m2"]))

    run_sweep(ap, s1_mm, te_d, "g0", "b0", pf1)
    w16, w8 = nxt16, nxt8
    nxt16, nxt8 = {}, {}

    # ---- Sweep 2: text stage 2 -> ot, a2t_d -----------------------------
    def s2_mm(b):
        te16 = ap.tile([128, C, NB], F16, tag="x16", bufs=2, name="te16")
        nc.sync.dma_start(te16[:], te_d[bsl(b)])
        te8 = ap.tile([128, C, NB], FP8, tag="sa8", bufs=2, name="te8")
        nc.scalar.activation(te8[:], te16[:], AF.Copy, bias=0.0, scale=XS)
        a8 = ap.tile([128, C, NB], FP8, tag="x8a", bufs=2, name="a8")
        nc.sync.dma_start(a8[:], xa8[bsl(b)])
        Q = ap.tile([128, C, NB], F16, tag="bige", bufs=2, name="Q")
        emit16(w16["w3"], te16, act_evict(Q, "q"))
        nc.sync.dma_start(a2t_d[bsl(b)], Q[:])
        y = ap.tile([128, C, NB], F16, tag="y", bufs=3, name="y")
        emit8([(w8["w2"], a8)], stt_evict(y, "k2", Q))
        g = ap.tile([128, C, NB], F16, tag="g", bufs=1, name="g")
        emit8([(w8["m2"], te8)], sig_evict(g, "km2", "cm2"))
        nc.vector.tensor_mul(y[:], g[:], y[:])
        nc.vector.tensor_add(y[:], y[:], te16[:])
        yb8, ysq8 = make_stats_in(ap, y, "")
        return y, yb8, ysq8

    def pf2():
        nxt8.update(load_w8(["w5", "w4", "m3", "g1bw1"]))

    run_sweep(ap, s2_mm, ot, "g0", "b0", pf2)
    w8 = nxt8
    nxt8 = {}

    # ---- Sweep 3: vision -> ov, a2v_d -----------------------------------
    def s3_mm(b):
        v16 = ap.tile([128, C, NB], F16, tag="x16", bufs=2, name="v16")
        nc.sync.dma_start(v16[:], xv16[bsl(b)])
        v8 = ap.tile([128, C, NB], FP8, tag="x8a", bufs=2, name="v8")
        nc.sync.dma_start(v8[:], xv8[bsl(b)])
        t8 = ap.tile([128, C, NB], FP8, tag="sa8", bufs=2, name="t8")
        nc.sync.dma_start(t8[:], xt8[bsl(b)])
        a8 = ap.tile([128, C, NB], FP8, tag="x8c", bufs=1, name="a8")
        nc.sync.dma_start(a8[:], xa8[bsl(b)])
        P = ap.tile([128, C, NB], F16, tag="bige", bufs=2, name="P")
        nc.sync.dma_start(P[:], v2t_d[bsl(b)])
        Rt = ap.tile([128, C, NB], F16, tag="bige", bufs=2, name="Rt")
        emit8([(w8["w5"], v8)], act_evict(Rt, "r", "k5"))
        y = ap.tile([128, C, NB], F16, tag="y", bufs=3, name="y")
        emit8([(w8["w4"], a8)], stt_evict(y, "k4", Rt))
        g = ap.tile([128, C, NB], F16, tag="g", bufs=1, name="g")
        emit8([(w8["m3"], v8), (w8["g1bw1"], t8)],
              sig_evict(g, "km3", "cm3t"))
        # audio's gated sum sa = a2t + a2v + delta: computed here where PE
        # has headroom, streamed to DRAM for sweep 4
        Q = ap.tile([128, C, NB], F16, tag="sa", bufs=2, name="Q")
        nc.sync.dma_start(Q[:], a2t_d[bsl(b)])
        sa = ap.tile([128, C, NB], F16, tag="sa", bufs=2, name="sa")
        for ci in range(C):
            nc.vector.scalar_tensor_tensor(
                sa[:, ci, :], Q[:, ci, :], vs("delta", ci), Rt[:, ci, :],
                OP.add, OP.add)
        nc.sync.dma_start(sa16_d[bsl(b)], sa[:])
        sa8 = ap.tile([128, C, NB], FP8, tag="sa8", bufs=2, name="sa8")
        nc.scalar.activation(sa8[:], sa[:], AF.Copy, bias=0.0, scale=XS)
        nc.sync.dma_start(sa8_d[bsl(b)], sa8[:])
        nc.vector.tensor_add(y[:], y[:], P[:])
        nc.vector.tensor_mul(y[:], g[:], y[:])
        nc.vector.tensor_add(y[:], y[:], v16[:])
        yb8, ysq8 = make_stats_in(ap, y, "")
        return y, yb8, ysq8

    def pf3():
        nxt8.update(load_w8(["g2a", "g2b"]))

    run_sweep(ap, s3_mm, ov, "g1", "b1", pf3)
    w8 = nxt8

    # ---- Sweep 4: audio -> oa -------------------------------------------
    def s4_mm(b):
        a16 = ap.tile([128, C, NB], F16, tag="x16", bufs=2, name="a16")
        nc.sync.dma_start(a16[:], xa16[bsl(b)])
        a8 = ap.tile([128, C, NB], FP8, tag="x8a", bufs=2, name="a8")
        nc.sync.dma_start(a8[:], xa8[bsl(b)])
        sa = ap.tile([128, C, NB], F16, tag="sa", bufs=2, name="sa")
        nc.sync.dma_start(sa[:], sa16_d[bsl(b)])
        sa8 = ap.tile([128, C, NB], FP8, tag="sa8", bufs=2, name="sa8")
        nc.sync.dma_start(sa8[:], sa8_d[bsl(b)])
        g = ap.tile([128, C, NB], F16, tag="g", bufs=1, name="g")
        emit8([(w8["g2a"], a8), (w8["g2b"], sa8)],
              sig_evict(g, "kg2", "cga"))
        y = ap.tile([128, C, NB], F16, tag="y", bufs=3, name="y")
        nc.vector.tensor_mul(y[:], g[:], sa[:])
        nc.vector.tensor_add(y[:], y[:], a16[:])
        yb8, ysq8 = make_stats_in(ap, y, "")
        return y, yb8, ysq8

    run_sweep(ap, s4_mm, oa, "g2", "b2", None)


# ---------------------------------------------------------------------------
# Host side
# ---------------------------------------------------------------------------

def _pack_act(x, dtype, scale=1.0):
    """[rows, 1024] f32 -> [128, C, rows] (transposed, chunked)."""
    r = x.shape[0]
    v = x.T.reshape(C, 128, r).transpose(1, 0, 2)
    if scale != 1.0:
        v = np.clip(v * scale, -240.0, 240.0)
    return np.ascontiguousarray(v).astype(dtype)


def _pack_w(m, dtype, scale=1.0):
    """W [1024(out), 1024(in)] -> lhsT [128, C(kc), 1024(out)]."""
    v = m.reshape(DIM, C, 128).transpose(2, 1, 0)
    if scale != 1.0:
        v = np.clip(v * scale, -240.0, 240.0)
    return np.ascontiguousarray(v).astype(dtype)


def _unpack_out(o):
    """[128, C, rows] f16 -> [rows, 1024] f32."""
    r = o.shape[2]
    return np.ascontiguousarray(
        o.transpose(1, 0, 2).astype(np.float32).reshape(DIM, r).T)


def _pow2_scale(m, target=192.0):
    return float(2.0 ** np.floor(np.log2(target / np.abs(m).max())))


_PROG = {}


def _get_prog(simple_ln):
    if simple_ln not in _PROG:
        _PROG[simple_ln] = build_program(simple_ln=simple_ln)
    return _PROG[simple_ln]


def fold_weights(mha_w_in, mha_b_in, mha_w_out, mha_b_out, gate_w, gate_b):
    W, c = [], []
    for i in range(6):
        w_v = mha_w_in[i][2 * DIM:3 * DIM]
        b_v = mha_b_in[i][2 * DIM:3 * DIM]
        W.append(mha_w_out[i] @ w_v)
        c.append(mha_w_out[i] @ b_v + mha_b_out[i])
    Ga = [gate_w[j][:, :DIM] for j in range(3)]
    Gb = [gate_w[j][:, DIM:] for j in range(3)]
    mats = {
        "w1": W[1], "w3": W[3], "w5": W[5],
        "w0": W[0], "w2": W[2], "w4": W[4],
        "m1": Ga[0] + Gb[0] @ W[1],
        "m2": Ga[0] + Gb[0] @ W[3],
        "m3": Ga[1] + Gb[1] @ W[5],
        "g1bw1": Gb[1] @ W[1],
        "g2a": Ga[2], "g2b": Gb[2],
    }
    cvecs = {
        "p": c[0] + c[1],
        "q": c[2] + c[3],
        "r": c[4] + c[5] - c[0],
        "delta": c[0] - c[2] - c[4],
        "cm1": gate_b[0] + Gb[0] @ c[1],
        "cm2": gate_b[0] + Gb[0] @ c[3],
        "cm3t": gate_b[1] + Gb[1] @ c[5] + Gb[1] @ c[1],
        "cga": gate_b[2],
    }
    return mats, cvecs


LAST_EXEC_TIME_NS = None


def timed_run(inputs):
    """Re-run the kernel with NTFF tracing; returns HW exec time in ns."""
    kernel(**inputs, _trace=True)
    return LAST_EXEC_TIME_NS


def kernel(text, vision, audio, mha_w_in, mha_b_in, mha_w_out, mha_b_out,
           gate_w, gate_b, ln_scale, ln_bias, _trace=False):
    f32 = lambda a: np.asarray(a, dtype=np.float32)
    text, vision, audio = f32(text), f32(vision), f32(audio)
    mha_w_in, mha_b_in = f32(mha_w_in), f32(mha_b_in)
    mha_w_out, mha_b_out = f32(mha_w_out), f32(mha_b_out)
    gate_w, gate_b = f32(gate_w), f32(gate_b)
    ln_scale, ln_bias = f32(ln_scale), f32(ln_bias)

    simple_ln = bool(np.all(ln_scale == 1.0) and np.all(ln_bias == 0.0))
    nc = _get_prog(simple_ln)

    mats, cvecs = fold_weights(mha_w_in, mha_b_in, mha_w_out, mha_b_out,
                               gate_w, gate_b)
    # fp8 weight scales (shared within accumulation groups)
    sc = {n: _pow2_scale(mats[n]) for n in W8_NAMES}
    s3 = min(sc["m3"], sc["g1bw1"])
    sc["m3"] = sc["g1bw1"] = s3
    s4 = min(sc["g2a"], sc["g2b"])
    sc["g2a"] = sc["g2b"] = s4

    wdev = {n: _pack_w(mats[n], F16N) for n in W16_NAMES}
    wdev.update({n: _pack_w(mats[n], E4, sc[n]) for n in W8_NAMES})

    V = np.zeros((NVEC, DIM), np.float32)
    for n in ("p", "q", "r", "delta", "cm1", "cm2", "cm3t", "cga"):
        V[VEC_IDX[n]] = cvecs[n]
    for n, kn in [("w0", "k0"), ("w2", "k2"), ("w4", "k4"), ("w5", "k5"),
                  ("m1", "km1"), ("m2", "km2"), ("m3", "km3"), ("g2a", "kg2")]:
        V[VEC_IDX[kn]] = 1.0 / (XS * sc[n])
    for j, (gn, bn) in enumerate([("g0", "b0"), ("g1", "b1"), ("g2", "b2")]):
        V[VEC_IDX[gn]] = ln_scale[j]
        V[VEC_IDX[bn]] = ln_bias[j]
    vecs_dev = np.ascontiguousarray(
        V.reshape(NVEC, C, 128).transpose(2, 0, 1)).astype(np.float32)

    in_maps = []
    for cid in range(NCORES):
        sl = slice(cid * R, (cid + 1) * R)
        in_maps.append({
            "xt16": _pack_act(text[sl], F16N),
            "xv16": _pack_act(vision[sl], F16N),
            "xa16": _pack_act(audio[sl], F16N),
            "xt8": _pack_act(text[sl], E4, XS),
            "xv8": _pack_act(vision[sl], E4, XS),
            "xa8": _pack_act(audio[sl], E4, XS),
            "vecs": vecs_dev,
            **wdev,
        })

    # The device occasionally throws a transient NRT_EXEC_UNIT_UNRECOVERABLE
    # on the first execute; retry a couple of times before giving up.
    last_err = None
    for attempt in range(3):
        try:
            res = bass_utils.run_bass_kernel_spmd(
                nc, in_maps, core_ids=list(range(NCORES)), trace=_trace)
            break
        except Exception as e:
            last_err = e
            import time as _time
            _time.sleep(5)
    else:
        raise last_err
    if _trace:
        global LAST_EXEC_TIME_NS
        LAST_EXEC_TIME_NS = res.exec_time_ns
        if res.instructions_and_trace:
            print("trace:", res.instructions_and_trace[1])

    outs = {k: np.empty((BATCH, DIM), np.float32) for k in ("ot", "ov", "oa")}
    for cid in range(NCORES):
        sl = slice(cid * R, (cid + 1) * R)
        for k in outs:
            outs[k][sl] = _unpack_out(res.results[cid][k])
    return (outs["ot"], outs["ov"], outs["oa"])
